# revision 30
# baseline (speedup 1.0000x reference)
"""Trainium2 Bass kernel for nn_MultiHeadAttention_6786048328624 (sparse_attention).

Strategy (8 NeuronCores, data-parallel over batch B=8, one batch per core).

Math (equivalent to the reference in fp32; validated empirically):
  - scores computed TRANSPOSED per head: S^T[k,q] = Kh @ Qh^T so the
    attention-V contraction needs no transposes; an appended ones-column on V
    yields the softmax denominator Z[q] in psum row 64 for free.
  - softmax skips max-subtraction (scores/8 + bias bounded, exp safe in fp16).
  - mask folded additively: logb = w0*f(t) + w1*f(d) + b + (mask-1)*50;
    exp(logb) underflows to exactly 0 in fp16 where masked.
  - bias mats broadcast over heads: eb = exp(logb) computed once per batch,
    multiplied into exp(scores) per head.
  - bk cancels in softmax; bv/bo fold into a host-side constant row; bq==0.

v3 structure (vs the 256us baseline):
  - all inputs shipped fp16 from host (pure relayout/dtype conversion).
  - ONE activation table set (natural_log_exp_and_others) serves Ln and Exp:
    the per-build table info is filtered so every function resolves to that
    set -> a single ACT_TABLE_LOAD instead of 18.
  - bias-mat DMAs issued first so the ACT Ln chain starts immediately.
  - eb exps interleaved into the Ln stream (same table set, no switches).
  - eb duplicated side-by-side (SBUF->SBUF DMA) so each head-pair k-tile
    needs ONE [128,2048] fp16 2x multiply on DVE instead of two.
  - score matmuls for a head pair issued adjacently; lhsT base partitions
    (0/64) auto-derive disjoint PE row groups -> concurrent K=64 matmuls.
  - attnV lagged 3 k-tiles behind scores; previous pair's softmax-normalize
    (Z -> DRAM-bounce broadcast -> reciprocal -> multiply) emitted at the
    next pair's head so psum frees never head-of-line block the PE queue.
"""

import numpy as np
from contextlib import ExitStack

import concourse.bass as bass
import concourse.tile as tile
from concourse import bacc, mybir
from concourse.bass_utils import run_bass_kernel_spmd

F32 = mybir.dt.float32
F16 = mybir.dt.float16
AF = mybir.ActivationFunctionType
ALU = mybir.AluOpType

B, S, D, H, DK = 8, 1024, 512, 8, 64
NT = S // 128         # 8 k-tiles of 128
NC = D // 128         # 4 chunks of the model dim
MASK_NEG = 50.0

GP_MULT_KTS = (2, 4, 6)  # kts whose es*eb multiply runs on GPSIMD
ATTNV_LAG = 3         # attnV(kt) emitted after scores(kt+LAG)
EB_LAG = 3            # eb exp(kt) emitted after Ln(kt+LAG)

_COMBINED_SET = "natural_log_exp_and_others"
_tables_patched = False


def _patch_act_tables():
    """Make every activation function resolve to the combined Ln+Exp table
    set so the kernel needs exactly one ACT_TABLE_LOAD.  Set IDs are list
    positions, so ordering/names are preserved and only the *membership*
    used for selection is filtered."""
    global _tables_patched
    if _tables_patched:
        return
    orig = bacc.get_activation_tables

    def filtered(arch):
        t = orig(arch)
        return {name: (fns if name == _COMBINED_SET else frozenset())
                for name, fns in t.items()}

    bacc.get_activation_tables = filtered
    _tables_patched = True


def build_nc(w0=0.0, w1=0.0, bb=0.0, dbg=False):
    _patch_act_tables()
    nc = bacc.Bacc("TRN2", target_bir_lowering=False, debug=False)

    q_d = nc.dram_tensor("q16", [D, S], F16, kind="ExternalInput").ap()
    k_d = nc.dram_tensor("k16", [D, S], F16, kind="ExternalInput").ap()
    v_d = nc.dram_tensor("v16", [D, S], F16, kind="ExternalInput").ap()
    t_d = nc.dram_tensor("t16", [S, S], F16, kind="ExternalInput").ap()
    d_d = nc.dram_tensor("d16", [S, S], F16, kind="ExternalInput").ap()
    m_d = nc.dram_tensor("m16", [S, S], F16, kind="ExternalInput").ap()
    wq_d = nc.dram_tensor("Wq16", [D, D], F16, kind="ExternalInput").ap()
    wk_d = nc.dram_tensor("Wk16", [D, D], F16, kind="ExternalInput").ap()
    wv_d = nc.dram_tensor("Wv16", [D, D], F16, kind="ExternalInput").ap()
    wo_d = nc.dram_tensor("Wo16", [D, D], F16, kind="ExternalInput").ap()
    out_d = nc.dram_tensor("out16", [S, D], F16, kind="ExternalOutput").ap()

    with tile.TileContext(nc) as tc, ExitStack() as ctx:
        ctx.enter_context(nc.allow_low_precision(
            reason="fp16 hot path validated vs fp32 reference"))
        persist = ctx.enter_context(tc.tile_pool(name="persist", bufs=1))
        bload = ctx.enter_context(tc.tile_pool(name="bload", bufs=3))
        lpool = ctx.enter_context(tc.tile_pool(name="lpool", bufs=2))
        rpool = ctx.enter_context(tc.tile_pool(name="rpool", bufs=2))
        espool = ctx.enter_context(tc.tile_pool(name="espool", bufs=3))
        atpool = ctx.enter_context(tc.tile_pool(name="atpool", bufs=4))
        zpool = ctx.enter_context(tc.tile_pool(name="zpool", bufs=1))
        outsb = ctx.enter_context(tc.tile_pool(name="outsb", bufs=1))
        ps_s = ctx.enter_context(tc.tile_pool(name="ps_s", bufs=2, space="PSUM"))
        ps_o = ctx.enter_context(tc.tile_pool(name="ps_o", bufs=2, space="PSUM"))
        zdram = ctx.enter_context(tc.tile_pool(name="zdram", bufs=2, space="DRAM"))

        e_t = persist.tile([128, 1], F32, tag="e_t")
        nc.vector.memset(e_t[:], float(np.e))

        # ---- input DMAs: first two k-tiles of bias mats lead, so the ACT
        #      Ln chain starts ~immediately; weights/qkv next (projections);
        #      remaining bias tiles stream behind. ----
        tld, dld, mld = [None] * NT, [None] * NT, [None] * NT

        def load_bias_kt(kt):
            tl = bload.tile([128, S], F16, tag="tld", name=f"tld{kt}")
            nc.sync.dma_start(tl[:], t_d[kt * 128:(kt + 1) * 128, :])
            dl = bload.tile([128, S], F16, tag="dld", name=f"dld{kt}")
            nc.sync.dma_start(dl[:], d_d[kt * 128:(kt + 1) * 128, :])
            ml = bload.tile([128, S], F16, tag="mld", name=f"mld{kt}")
            nc.sync.dma_start(ml[:], m_d[kt * 128:(kt + 1) * 128, :])
            tld[kt], dld[kt], mld[kt] = tl, dl, ml

        for kt in range(3):
            load_bias_kt(kt)

        def load_w(dram, name):
            tiles = []
            for c in range(NC):
                w16 = persist.tile([128, D], F16, tag=f"{name}{c}",
                                   name=f"{name}{c}")
                nc.sync.dma_start(w16[:], dram[c * 128:(c + 1) * 128, :])
                tiles.append(w16)
            return tiles

        wq16 = load_w(wq_d, "wq")
        wk16 = load_w(wk_d, "wk")
        wv16 = load_w(wv_d, "wv")
        wo16 = load_w(wo_d, "wo")

        def load_x(dram, name):
            xs = []
            for kc in range(NC):
                x16 = persist.tile([128, S], F16, tag=f"{name}{kc}",
                                   name=f"{name}{kc}")
                nc.sync.dma_start(x16[:], dram[kc * 128:(kc + 1) * 128, :])
                xs.append(x16)
            return xs

        xq = load_x(q_d, "xq")
        xk = load_x(k_d, "xk")
        xv = load_x(v_d, "xv")

        for kt in range(3, NT):
            load_bias_kt(kt)

        # ---- bias chain; eb exps interleave into the Ln stream lagged by
        #      EB_LAG k-tiles (same ACT table set -> no switch cost).
        #      logb tiles reuse the xq/xk slots (dead after projections). ----
        LOGB = [None] * NT
        EB2 = [None] * NT

        def emit_eb(kt):
            eb = persist.tile([128, 2 * S], F16, tag=f"eb{kt}",
                              name=f"eb{kt}")
            nc.scalar.activation(eb[:, 0:S], LOGB[kt][:], AF.Exp)
            nc.sync.dma_start(eb[:, S:2 * S], eb[:, 0:S])
            EB2[kt] = eb

        for kt in range(NT):
            L = lpool.tile([128, 2 * S], F32, tag="L", name=f"L{kt}")
            nc.scalar.activation(L[:, 0:S], tld[kt][:], AF.Ln, bias=e_t[:],
                                 scale=100.0)
            nc.scalar.activation(L[:, S:2 * S], dld[kt][:], AF.Ln,
                                 bias=e_t[:], scale=100.0)
            mterm = rpool.tile([128, S], F16, tag="mt", name=f"mt{kt}")
            nc.gpsimd.tensor_scalar(mterm[:], mld[kt][:], MASK_NEG,
                                    bb - MASK_NEG, ALU.mult, ALU.add)
            R = rpool.tile([128, 2 * S], F32, tag="R", name=f"R{kt}")
            nc.vector.reciprocal_approx_fast(R[:], L[:])
            tmp = lpool.tile([128, S], F32, tag="tmp", name=f"tmp{kt}",
                             bufs=1)
            nc.vector.scalar_tensor_tensor(tmp[:], R[:, S:2 * S], w1,
                                           mterm[:], ALU.mult, ALU.add)
            xt = "xq" if kt < 4 else "xk"
            lg = persist.tile([128, S], F16, tag=f"{xt}{kt % 4}",
                              name=f"logb{kt}")
            nc.vector.scalar_tensor_tensor(lg[:], R[:, 0:S], w0, tmp[:],
                                           ALU.mult, ALU.add)
            LOGB[kt] = lg
            if kt >= EB_LAG:
                emit_eb(kt - EB_LAG)
        for kt in range(NT - EB_LAG, NT):
            emit_eb(kt)

        # ---- q/k projections (PE runs these during the bias chain) ----
        QT16, KT16 = [], []
        for w16, xs, name, dst in ((wq16, xq, "qt", QT16),
                                   (wk16, xk, "kt", KT16)):
            for c in range(NC):
                ps = ps_s.tile([128, S], F32, tag="sps", name=f"ps_{name}{c}")
                for kc in range(NC):
                    for j in range(2):
                        nc.tensor.matmul(
                            ps[:, j * 512:(j + 1) * 512],
                            w16[kc][:, c * 128:(c + 1) * 128],
                            xs[kc][:, j * 512:(j + 1) * 512],
                            start=(kc == 0), stop=(kc == NC - 1),
                            skip_group_check=True)
                t16 = persist.tile([128, S], F16, tag=f"{name}{c}",
                                   name=f"{name}{c}")
                nc.vector.tensor_copy(t16[:], ps[:])
                dst.append(t16)

        # ---- v projection -> [128, H, 65] per k-tile (ones col -> Z) ----
        V_sb = []
        for st in range(NT):
            ps = ps_o.tile([128, D], F32, tag="ot", name=f"ps_v{st}")
            for kc in range(NC):
                nc.tensor.matmul(ps[:], xv[kc][:, st * 128:(st + 1) * 128],
                                 wv16[kc][:], start=(kc == 0),
                                 stop=(kc == NC - 1), skip_group_check=True)
            vt = persist.tile([128, H, 65], F16, tag=f"v{st}", name=f"v{st}")
            nc.vector.tensor_copy(
                vt[:, :, 0:64], ps.rearrange("p (h d) -> p h d", h=H))
            nc.gpsimd.memset(vt[:, :, 64:65], 1.0)
            V_sb.append(vt)

        # ---- attention ----
        OutP = [persist.tile([128, S], F16, tag=f"op{c}", name=f"op{c}")
                for c in range(NC)]

        dbg_ot_d = dbg_zb_d = None
        if dbg:
            dbg_ot_d = nc.dram_tensor("dbg_ot", [NC * 2 * 65, S], F32,
                                      kind="ExternalOutput").ap()
            dbg_zb_d = nc.dram_tensor("dbg_zb", [NC * 64, 2 * S], F32,
                                      kind="ExternalOutput").ap()

        def z_chain_steps(c, ots):
            """Normalize pair c in two steps that interleave into the next
            pair's k-tile loop (keeps the DVE queue from head-of-line
            blocking on the bounce DMA round-trip)."""
            zs = zpool.tile([65, 2 * S], F32, tag="zs", name=f"zs{c}")
            zbb = zpool.tile([64, 2 * S], F32, tag="zbb", name=f"zbb{c}")

            def step_a():
                if dbg:
                    for hh in range(2):
                        otf = zpool.tile([65, S], F32, tag=f"dbgot{hh}",
                                         name=f"dbgot{c}_{hh}")
                        nc.vector.tensor_copy(otf[:], ots[hh][:])
                        nc.sync.dma_start(
                            dbg_ot_d[(2 * c + hh) * 65:
                                     (2 * c + hh + 1) * 65, :], otf[:])
                nc.vector.tensor_copy(zs[64:65, 0:S], ots[0][64:65, :])
                nc.vector.tensor_copy(zs[64:65, S:2 * S], ots[1][64:65, :])
                zd = zdram.tile([1, 2 * S], F32, tag="zd", name=f"zd{c}")
                nc.sync.dma_start(zd[:], zs[64:65, :])
                for hh in range(2):
                    nc.sync.dma_start(
                        zbb[:, hh * S:(hh + 1) * S],
                        bass.AP(tensor=zd.tensor, offset=zd.offset + hh * S,
                                ap=[[0, 64], [1, S]]))

            def step_b():
                # zs is dead once zd was written; reuse its slot for zb
                zb = zpool.tile([64, 2 * S], F32, tag="zs", name=f"zb{c}")
                nc.vector.reciprocal_approx_fast(zb[:], zbb[:])
                if dbg:
                    nc.sync.dma_start(dbg_zb_d[c * 64:(c + 1) * 64, :],
                                      zb[:])
                nc.vector.tensor_tensor(OutP[c][0:64, :], ots[0][0:64, :],
                                        zb[:, 0:S], op=ALU.mult)
                o16 = zpool.tile([64, S], F16, tag="zbb", name=f"o16_{c}")
                nc.vector.tensor_tensor(o16[:], ots[1][0:64, :],
                                        zb[:, S:2 * S], op=ALU.mult)
                nc.sync.dma_start(OutP[c][64:128, :], o16[:])

            return [step_a, step_b]

        zjobs = []  # pending normalize steps from the previous pair
        for c in range(NC):
            ots = [ps_o.tile([65, S], F32, tag="ot", name=f"ot{c}_{hh}")
                   for hh in range(2)]
            at2s = {}

            def emit_attnv(kt, c=c, ots=ots, at2s=at2s):
                at2 = at2s.pop(kt)
                for hh in range(2):
                    h = 2 * c + hh
                    for j in range(2):
                        nc.tensor.matmul(
                            ots[hh][:, j * 512:(j + 1) * 512],
                            V_sb[kt][:, h, :],
                            at2[:, hh * S + j * 512:hh * S + (j + 1) * 512],
                            start=(kt == 0), stop=(kt == NT - 1),
                            skip_group_check=True)

            for kt in range(NT):
                sps = []
                for hh in range(2):
                    sp = ps_s.tile([128, S], F32, tag="sps",
                                   name=f"sps{c}_{kt}_{hh}")
                    sps.append(sp)
                # adjacent issues, disjoint row groups -> concurrent in PE
                for j in range(2):
                    for hh in range(2):
                        kh = KT16[c][hh * 64:(hh + 1) * 64,
                                     kt * 128:(kt + 1) * 128]
                        qh = QT16[c][hh * 64:(hh + 1) * 64,
                                     j * 512:(j + 1) * 512]
                        nc.tensor.matmul(sps[hh][:, j * 512:(j + 1) * 512],
                                         kh, qh, start=True, stop=True,
                                         skip_group_check=True)
                es2 = espool.tile([128, 2 * S], F16, tag="es",
                                  name=f"es{c}_{kt}")
                for hh in range(2):
                    nc.scalar.activation(es2[:, hh * S:(hh + 1) * S],
                                         sps[hh][:], AF.Exp, scale=1.0 / 8.0)
                at2 = atpool.tile([128, 2 * S], F16, tag="at",
                                  name=f"at{c}_{kt}")
                eng = nc.gpsimd if kt in GP_MULT_KTS else nc.vector
                eng.tensor_tensor(at2[:], es2[:], EB2[kt][:], op=ALU.mult)
                at2s[kt] = at2
                if kt in (0, 2) and zjobs:
                    zjobs.pop(0)()
                if kt >= ATTNV_LAG:
                    emit_attnv(kt - ATTNV_LAG)
            for kt in range(NT - ATTNV_LAG, NT):
                emit_attnv(kt)
            zjobs = z_chain_steps(c, ots)
        for step in zjobs:
            step()

        if dbg:
            dbg_eb = nc.dram_tensor("dbg_eb", [NT * 128, S], F16,
                                    kind="ExternalOutput").ap()
            dbg_logb = nc.dram_tensor("dbg_logb", [NT * 128, S], F16,
                                      kind="ExternalOutput").ap()
            dbg_qt = nc.dram_tensor("dbg_qt", [D, S], F16,
                                    kind="ExternalOutput").ap()
            dbg_kt = nc.dram_tensor("dbg_kt", [D, S], F16,
                                    kind="ExternalOutput").ap()
            dbg_v = nc.dram_tensor("dbg_v", [NT * 128, H * 65], F16,
                                   kind="ExternalOutput").ap()
            dbg_outp = nc.dram_tensor("dbg_outp", [NC * 128, S], F16,
                                      kind="ExternalOutput").ap()
            for kt in range(NT):
                nc.sync.dma_start(dbg_eb[kt * 128:(kt + 1) * 128, :],
                                  EB2[kt][:, 0:S])
                nc.sync.dma_start(dbg_logb[kt * 128:(kt + 1) * 128, :],
                                  LOGB[kt][:])
                nc.sync.dma_start(
                    dbg_v[kt * 128:(kt + 1) * 128, :],
                    V_sb[kt].rearrange("p h d -> p (h d)"))
            for c in range(NC):
                nc.sync.dma_start(dbg_qt[c * 128:(c + 1) * 128, :],
                                  QT16[c][:])
                nc.sync.dma_start(dbg_kt[c * 128:(c + 1) * 128, :],
                                  KT16[c][:])
                nc.sync.dma_start(dbg_outp[c * 128:(c + 1) * 128, :],
                                  OutP[c][:])

        # ---- output projection: accumulate head pairs, K=128 each ----
        for st in range(NT):
            f = ps_o.tile([128, D], F32, tag="ot", name=f"f{st}")
            for p in range(NC):
                nc.tensor.matmul(f[:], OutP[p][:, st * 128:(st + 1) * 128],
                                 wo16[p][:], start=(p == 0),
                                 stop=(p == NC - 1), skip_group_check=True)
            o = outsb.tile([128, D], F16, tag="o", name=f"o{st}")
            nc.scalar.copy(o[:], f[:])
            nc.sync.dma_start(out_d[st * 128:(st + 1) * 128, :], o[:])

    nc.compile()
    return nc


_NC = None


def make_in_maps(q, k, v, temporal_mat, dis_mat, mask, Wq, Wk, Wv, Wo,
                 w_bias=None, b_bias=None):
    in_maps = []
    for b in range(B):
        in_maps.append({
            "q16": np.ascontiguousarray(q[b].T).astype(np.float16),
            "k16": np.ascontiguousarray(k[b].T).astype(np.float16),
            "v16": np.ascontiguousarray(v[b].T).astype(np.float16),
            "t16": np.ascontiguousarray(temporal_mat[b].T).astype(np.float16),
            "d16": np.ascontiguousarray(dis_mat[b].T).astype(np.float16),
            "m16": np.ascontiguousarray(mask[b].T).astype(np.float16),
            "Wq16": Wq.astype(np.float16), "Wk16": Wk.astype(np.float16),
            "Wv16": Wv.astype(np.float16), "Wo16": Wo.astype(np.float16),
        })
    return in_maps


def kernel(q, k, v, temporal_mat, dis_mat, mask,
           Wq, bq, Wk, bk, Wv, bv, w_bias, b_bias, Wo, bo):
    global _NC
    q = np.asarray(q, np.float32)
    k = np.asarray(k, np.float32)
    v = np.asarray(v, np.float32)
    temporal_mat = np.asarray(temporal_mat, np.float32)
    dis_mat = np.asarray(dis_mat, np.float32)
    mask = np.asarray(mask, np.int32)
    Wq, Wk, Wv, Wo = (np.asarray(x, np.float32) for x in (Wq, Wk, Wv, Wo))
    w_bias = np.asarray(w_bias, np.float32)
    b_bias = float(np.asarray(b_bias, np.float32).reshape(()))

    # bk cancels exactly in softmax; bv/bo fold into a constant output row
    # added after the gather; bq must be zero (it is in setup_inputs).
    assert np.allclose(np.asarray(bq), 0.0), "nonzero bq unsupported"
    bo_eff = np.asarray(bv, np.float32) @ Wo + np.asarray(bo, np.float32)

    if _NC is None:
        _NC = build_nc(float(w_bias[0]), float(w_bias[1]), b_bias)

    in_maps = make_in_maps(q, k, v, temporal_mat, dis_mat, mask,
                           Wq, Wk, Wv, Wo)
    res = run_bass_kernel_spmd(_NC, in_maps, core_ids=list(range(B)))
    out = np.stack([np.asarray(r["out16"], np.float32) for r in res.results],
                   axis=0)
    if np.any(bo_eff != 0.0):
        out = out + bo_eff[None, None, :]
    return out.astype(np.float32)
